# revision 17
# baseline (speedup 1.0000x reference)
"""Euclidean distance layer on 8 Trainium2 NeuronCores.

out[b, o] = || x[b, :] - weight[:, o] ||_2
x: [512, 256] f32, weight: [256, 1024] f32 -> out: [512, 1024] f32

Sharding: tensor-parallel over output features (8 x 128 columns per core).

Per core:  dist^2 = -2 * ( x~@w~_loc - 0.5*||w~_loc||^2 ) + ||x||^2
where x~, w~ are bf16 roundings of x, w (the x.w and ||w||^2 terms tolerate
bf16: combined ~2e-5 relative error on dist; ||x||^2 stays fp32).
  - x~@w~ on the PE into one fp32 PSUM bank [128, 4x128]
  - ||w~||^2: bf16 squares (DVE) -> [-0.25] x2-column reduce matmul ->
    stride-0 broadcast row copy -> K=2 bf16 fold matmuls
  - ||x||^2 per-partition columns, split: batch tiles 0/1 via ACT Square
    with accum_out, tiles 2/3 via DVE mul+reduce
  - final: out = sqrt(-2 * psum + ||x||^2) on ACT (affine scale + bias)
Raw bacc: manual semaphores. Host work is layout/dtype prep only:
transpose+round x, slice+round w, concat output slices.
"""

from contextlib import ExitStack

import numpy as np

B = 512      # batch
K = 256      # inputSize (contraction dim)
NOUT = 1024  # outputSize
NCORES = 8
NLOC = NOUT // NCORES  # 128 output features per core
P = 128                # partitions
KT = K // P            # 2 contraction chunks
MT = B // P            # 4 batch tiles

_NC = None  # cached compiled Bass program (same SPMD program on all cores)


def _build():
    import concourse.bass as bass
    from concourse import bacc, mybir

    f32 = mybir.dt.float32
    bf16 = mybir.dt.bfloat16
    Sqrt = mybir.ActivationFunctionType.Sqrt
    Square = mybir.ActivationFunctionType.Square
    ts = bass.ts

    nc = bacc.Bacc(
        "TRN2", target_bir_lowering=False, debug=False, num_devices=NCORES
    )

    xtb = nc.dram_tensor("xtb", [K, B], bf16, kind="ExternalInput")
    xn = nc.dram_tensor("xn", [B, K], f32, kind="ExternalInput")
    wlb = nc.dram_tensor("wlb", [K, NLOC], bf16, kind="ExternalInput")
    out = nc.dram_tensor("out", [B, NLOC], f32, kind="ExternalOutput")

    with ExitStack() as ctx:
        e = ctx.enter_context
        xtb_sb = e(nc.sbuf_tensor("xtbs", [P, KT, B], bf16))
        wlb_sb = e(nc.sbuf_tensor("wlbs", [P, KT, NLOC], bf16))
        xn_sb = [e(nc.sbuf_tensor(f"xns{h}", [P, 2, K], f32)) for h in range(2)]
        wlsq = [e(nc.sbuf_tensor(f"wlsq{k}", [P, NLOC], bf16)) for k in range(KT)]
        xsq_scrA = e(nc.sbuf_tensor("xsqsA", [P, 2, K], f32))
        xsq_scrD = e(nc.sbuf_tensor("xsqsD", [P, 2, K], f32))
        xsq_colA = e(nc.sbuf_tensor("xsqcA", [P, 2], f32))
        xsq_colD = e(nc.sbuf_tensor("xsqcD", [P, 2], f32))
        neg_q = e(nc.sbuf_tensor("neg_q", [P, 2], bf16))
        ones_m = e(nc.sbuf_tensor("ones_m", [2, P], bf16))
        wsq_row4 = e(nc.sbuf_tensor("wsq_row4", [2, MT, NLOC], bf16))
        out_sb = e(nc.sbuf_tensor("out_sb", [P, MT, NLOC], f32))

        ps_w = e(nc.psum_tensor("ps_w", [2, NLOC], f32))   # -0.25*||w||^2 x2
        ps_all = e(nc.psum_tensor("ps_all", [P, MT, NLOC], f32))  # one bank

        s_wl = e(nc.semaphore("s_wl"))
        s_xt = e(nc.semaphore("s_xt"))
        s_xn = [e(nc.semaphore(f"s_xn{h}")) for h in range(2)]
        s_sq = e(nc.semaphore("s_sq"))      # 2 = both wlsq done
        s_mm = e(nc.semaphore("s_mm"))      # 1 = wsq reduce, 2+m = aug m
        s_brd = e(nc.semaphore("s_brd"))    # 1 = wsq_row4 broadcast ready
        s_colD = e(nc.semaphore("s_colD"))  # 1 = xsq cols for m2/m3 ready
        s_sqrt = e(nc.semaphore("s_sqrt"))  # m+1 = sqrt tile m in out_sb
        s_out = e(nc.semaphore("s_out"))    # 16 = sync output DMA landed
        s_out2 = e(nc.semaphore("s_out2"))  # 16 = gpsimd output DMA landed
        block = e(nc.Block())

        @block.sync
        def _(sync):
            sync.dma_start(
                out=wlb_sb[:, :, :],
                in_=wlb[:, :].rearrange("(c p) o -> p c o", p=P),
            ).then_inc(s_wl, 16)
            sync.dma_start(
                out=xn_sb[0][:, :, :],
                in_=xn[0 : 2 * P, :].rearrange("(c p) k -> p c k", p=P),
            ).then_inc(s_xn[0], 16)
            sync.dma_start(
                out=xtb_sb[:, :, :],
                in_=xtb[:, :].rearrange("(c p) b -> p c b", p=P),
            ).then_inc(s_xt, 16)
            sync.wait_ge(s_sqrt, 2)
            sync.dma_start(
                out=out[0 : 2 * P, :].rearrange("(m p) o -> p m o", p=P),
                in_=out_sb[:, 0:2, :],
            ).then_inc(s_out, 16)
            sync.wait_ge(s_out, 16)

        @block.gpsimd
        def _(gpsimd):
            gpsimd.dma_start(
                out=xn_sb[1][:, :, :],
                in_=xn[2 * P : 4 * P, :].rearrange("(c p) k -> p c k", p=P),
            ).then_inc(s_xn[1], 16)
            gpsimd.wait_ge(s_sqrt, 4)
            gpsimd.dma_start(
                out=out[2 * P : 4 * P, :].rearrange("(m p) o -> p m o", p=P),
                in_=out_sb[:, 2:4, :],
            ).then_inc(s_out2, 16)
            gpsimd.wait_ge(s_out2, 16)

        @block.scalar
        def _(scalar):
            # ||x||^2 for batch tiles 0/1 (fp32, Square + free-dim accum)
            scalar.wait_ge(s_xn[0], 16)
            for m in range(2):
                scalar.activation(
                    xsq_scrA[:, m, :], xn_sb[0][:, m, :], Square,
                    accum_out=xsq_colA[:, m : m + 1],
                )
            scalar.drain()  # ACT RAW: sqrts below read xsq_colA
            for m in range(MT):
                scalar.wait_ge(s_mm, 2 + m)
                if m >= 2:
                    scalar.wait_ge(s_colD, 1)
                bias = (
                    xsq_colA[:, m : m + 1] if m < 2
                    else xsq_colD[:, m - 2 : m - 1]
                )
                scalar.activation(
                    out_sb[:, m, :], ps_all[:, m, :], Sqrt,
                    bias=bias, scale=-2.0,
                ).then_inc(s_sqrt)

        @block.vector
        def _(vector):
            vector.memset(neg_q[:, :], -0.25)
            vector.memset(ones_m[:, :], 1.0)
            vector.wait_ge(s_wl, 16)
            vector.tensor_mul(wlsq[0][:, :], wlb_sb[:, 0, :], wlb_sb[:, 0, :])
            vector.tensor_mul(
                wlsq[1][:, :], wlb_sb[:, 1, :], wlb_sb[:, 1, :]
            ).then_inc(s_sq, 2)
            vector.wait_ge(s_mm, 1)
            vector.tensor_copy(
                wsq_row4[:, :, :],
                bass.AP(tensor=ps_w, offset=0, ap=[[NLOC, 2], [0, MT], [1, NLOC]]),
            ).then_inc(s_brd)
            # ||x||^2 for batch tiles 2/3 (fp32 mul + reduce)
            vector.wait_ge(s_xn[1], 16)
            for m in range(2):
                vector.tensor_mul(
                    xsq_scrD[:, m, :], xn_sb[1][:, m, :], xn_sb[1][:, m, :]
                )
            vector.drain()  # DVE RAW: reduces read xsq_scrD
            for m in range(2):
                inst = vector.tensor_reduce(
                    xsq_colD[:, m : m + 1], xsq_scrD[:, m, :],
                    axis=mybir.AxisListType.X, op=mybir.AluOpType.add,
                )
            inst.then_inc(s_colD)

        @block.tensor
        def _(tensor):
            # -0.25*||w||^2 reduce, two identical rows
            tensor.wait_ge(s_sq, 2)
            tensor.matmul(
                ps_w[:, :], lhsT=neg_q[:, :], rhs=wlsq[0][:, :],
                start=True, stop=False,
            )
            tensor.matmul(
                ps_w[:, :], lhsT=neg_q[:, :], rhs=wlsq[1][:, :],
                start=False, stop=True,
            ).then_inc(s_mm)  # = 1
            # main bf16 matmuls: one PSUM bank, single start on the first
            tensor.wait_ge(s_xt, 16)
            for k in range(KT):
                for m in range(MT):
                    tensor.matmul(
                        ps_all[:, m, :],
                        lhsT=xtb_sb[:, k, ts(m, P)],
                        rhs=wlb_sb[:, k, :],
                        start=(k == 0 and m == 0), stop=False,
                        skip_group_check=True,
                    )
            # fold -0.5*||w||^2 per m-slice (K=2: two -0.25 rows)
            tensor.wait_ge(s_brd, 1)
            for m in range(MT):
                tensor.matmul(
                    ps_all[:, m, :],
                    lhsT=ones_m[:, :],
                    rhs=wsq_row4[:, m, :],
                    start=False, stop=True, skip_group_check=True,
                ).then_inc(s_mm)  # = 2 + m

    nc.compile()
    return nc


def _get_nc():
    global _NC
    if _NC is None:
        _NC = _build()
    return _NC


def _make_in_maps(x: np.ndarray, weight: np.ndarray):
    import ml_dtypes

    bf = ml_dtypes.bfloat16
    x = np.ascontiguousarray(x.astype(np.float32, copy=False))
    xtb = np.ascontiguousarray(x.T.astype(bf))
    wb = weight.astype(bf)
    return [
        {
            "xtb": xtb,
            "xn": x,
            "wlb": np.ascontiguousarray(wb[:, c * NLOC : (c + 1) * NLOC]),
        }
        for c in range(NCORES)
    ]


def run(x: np.ndarray, weight: np.ndarray, trace: bool = False):
    """Returns (full_output, BassKernelResults)."""
    from concourse.bass_utils import run_bass_kernel_spmd

    nc = _get_nc()
    res = run_bass_kernel_spmd(
        nc, _make_in_maps(x, weight), core_ids=list(range(NCORES)), trace=trace
    )
    full = np.concatenate(
        [res.results[c]["out"] for c in range(NCORES)], axis=1
    )
    return full, res


def kernel(x: np.ndarray, weight: np.ndarray) -> np.ndarray:
    return run(x, weight)[0]


# revision 26
# speedup vs baseline: 1.0512x; 1.0512x over previous
"""Euclidean distance layer on 8 Trainium2 NeuronCores.

out[b, o] = || x[b, :] - weight[:, o] ||_2
x: [512, 256] f32, weight: [256, 1024] f32 -> out: [512, 1024] f32

Sharding: tensor-parallel over output features (8 x 128 columns per core).

Per core:  dist^2 = -2 * ( x~@w~_loc - 0.5*||w~_loc||^2 ) + ||x||^2
where x~, w~ are fp16 roundings of x, w (the x.w and ||w||^2 terms tolerate
fp16 easily; ||x||^2 stays fp32 -> ~2e-5 relative error on dist).
  - x~@w~ on the PE into one fp32 PSUM bank [128, 4x128]
  - ||w~||^2: fp16 squares (DVE) -> [-0.25] x2-column reduce matmul ->
    stride-0 broadcast row copy -> K=2 fp16 fold matmuls
  - ||x||^2 per-partition fp32 columns, split: batch tiles 0/1 via ACT
    Square+accum_out, tiles 2/3 via DVE mul+reduce
  - final: out = sqrt(-2 * psum + ||x||^2) on ACT (affine scale + bias)
Raw bacc, manual semaphores. Input DMAs issue in the pre-block preamble.
Host work is layout/dtype prep only.
"""

from contextlib import ExitStack

import numpy as np

B = 512      # batch
K = 256      # inputSize (contraction dim)
NOUT = 1024  # outputSize
NCORES = 8
NLOC = NOUT // NCORES  # 128 output features per core
P = 128                # partitions
KT = K // P            # 2 contraction chunks
MT = B // P            # 4 batch tiles

_NC = None  # cached compiled Bass program (same SPMD program on all cores)


def _build():
    import concourse.bass as bass
    from concourse import bacc, mybir

    f32 = mybir.dt.float32
    f16 = mybir.dt.float16
    Sqrt = mybir.ActivationFunctionType.Sqrt
    Square = mybir.ActivationFunctionType.Square
    ts = bass.ts

    nc = bacc.Bacc(
        "TRN2", target_bir_lowering=False, debug=False, num_devices=NCORES
    )

    xtf = nc.dram_tensor("xtf", [K, B], f16, kind="ExternalInput")
    xn = nc.dram_tensor("xn", [B, K], f32, kind="ExternalInput")
    wlf = nc.dram_tensor("wlf", [K, NLOC], f16, kind="ExternalInput")
    out = nc.dram_tensor("out", [B, NLOC], f32, kind="ExternalOutput")

    with ExitStack() as ctx:
        e = ctx.enter_context
        xtf_sb = e(nc.sbuf_tensor("xtfs", [P, KT, B], f16))
        wlf_sb = e(nc.sbuf_tensor("wlfs", [P, KT, NLOC], f16))
        xn_sb = [e(nc.sbuf_tensor(f"xns{h}", [P, 2, K], f32)) for h in range(2)]
        wlsq = [e(nc.sbuf_tensor(f"wlsq{k}", [P, NLOC], f16)) for k in range(KT)]
        xsq_scrA = e(nc.sbuf_tensor("xsqsA", [P, 2, K], f32))
        xsq_scrD = e(nc.sbuf_tensor("xsqsD", [P, 2, K], f32))
        xsq_colA = e(nc.sbuf_tensor("xsqcA", [P, 2], f32))
        xsq_colD = e(nc.sbuf_tensor("xsqcD", [P, 2], f32))
        neg_q = e(nc.sbuf_tensor("neg_q", [P, 2], f16))
        ones_m = e(nc.sbuf_tensor("ones_m", [2, P], f16))
        wsq_row4 = e(nc.sbuf_tensor("wsq_row4", [2, MT, NLOC], f16))
        out_sb = e(nc.sbuf_tensor("out_sb", [P, MT, NLOC], f32))
        actwarm = e(nc.sbuf_tensor("actwarm", [1, 1], f32))

        ps_w = e(nc.psum_tensor("ps_w", [2, NLOC], f32))   # -0.25*||w||^2 x2
        ps_all = e(nc.psum_tensor("ps_all", [P, MT, NLOC], f32))  # one bank

        s_wl = e(nc.semaphore("s_wl"))
        s_xt = e(nc.semaphore("s_xt"))
        s_xn = [e(nc.semaphore(f"s_xn{h}")) for h in range(2)]
        s_sq = e(nc.semaphore("s_sq"))      # 2 = both wlsq done
        s_mm = e(nc.semaphore("s_mm"))      # 1 = wsq reduce, 2+m = aug m
        s_brd = e(nc.semaphore("s_brd"))    # 1 = wsq_row4 broadcast ready
        s_colD = e(nc.semaphore("s_colD"))  # 1 = xsq cols for m2/m3 ready
        s_sqrt = e(nc.semaphore("s_sqrt"))  # m+1 = sqrt tile m in out_sb
        s_out = e(nc.semaphore("s_out"))    # 16 = sync output DMA landed
        s_out2 = e(nc.semaphore("s_out2"))  # 16 = scalar output DMA landed

        block = e(nc.Block())

        @block.sync
        def _(sync):
            sync.dma_start(
                out=wlf_sb[:, :, :],
                in_=wlf[:, :].rearrange("(c p) o -> p c o", p=P),
            ).then_inc(s_wl, 16)
            sync.dma_start(
                out=xn_sb[0][:, :, :],
                in_=xn[0 : 2 * P, :].rearrange("(c p) k -> p c k", p=P),
            ).then_inc(s_xn[0], 16)
            sync.dma_start(
                out=xtf_sb[:, :, :],
                in_=xtf[:, :].rearrange("(c p) b -> p c b", p=P),
            ).then_inc(s_xt, 16)
            sync.wait_ge(s_sqrt, 2)
            sync.dma_start(
                out=out[0 : 2 * P, :].rearrange("(m p) o -> p m o", p=P),
                in_=out_sb[:, 0:2, :],
            ).then_inc(s_out, 16)
            sync.wait_ge(s_out, 16)

        @block.gpsimd
        def _(gpsimd):
            gpsimd.dma_start(
                out=xn_sb[1][:, :, :],
                in_=xn[2 * P : 4 * P, :].rearrange("(c p) k -> p c k", p=P),
            ).then_inc(s_xn[1], 16)
            gpsimd.wait_ge(s_sqrt, 4)
            gpsimd.dma_start(
                out=out[2 * P : 4 * P, :].rearrange("(m p) o -> p m o", p=P),
                in_=out_sb[:, 2:4, :],
            ).then_inc(s_out2, 16)
            gpsimd.wait_ge(s_out2, 16)

        @block.scalar
        def _(scalar):
            # ||x||^2 for batch tiles 0/1 (fp32, Square + free-dim accum)
            scalar.wait_ge(s_xn[0], 16)
            for m in range(2):
                scalar.activation(
                    xsq_scrA[:, m, :], xn_sb[0][:, m, :], Square,
                    accum_out=xsq_colA[:, m : m + 1],
                )
            scalar.drain()  # ACT RAW: sqrts below read xsq_colA
            for m in range(MT):
                scalar.wait_ge(s_mm, 2 + m)
                if m == 2:
                    scalar.wait_ge(s_colD, 1)
                bias = (
                    xsq_colA[:, m : m + 1] if m < 2
                    else xsq_colD[:, m - 2 : m - 1]
                )
                scalar.activation(
                    out_sb[:, m, :], ps_all[:, m, :], Sqrt,
                    bias=bias, scale=-2.0,
                ).then_inc(s_sqrt)


        @block.vector
        def _(vector):
            vector.memset(neg_q[:, :], -0.25)
            vector.memset(ones_m[:, :], 1.0)
            vector.wait_ge(s_wl, 16)
            vector.tensor_mul(wlsq[0][:, :], wlf_sb[:, 0, :], wlf_sb[:, 0, :])
            vector.tensor_mul(
                wlsq[1][:, :], wlf_sb[:, 1, :], wlf_sb[:, 1, :]
            ).then_inc(s_sq, 2)
            # ||x||^2 for batch tiles 2/3: fp32 squares
            vector.wait_ge(s_xn[1], 16)
            for m in range(2):
                vector.tensor_mul(
                    xsq_scrD[:, m, :], xn_sb[1][:, m, :], xn_sb[1][:, m, :]
                )
            # broadcast -0.25*||w||^2 rows across the 4 m-slices
            vector.wait_ge(s_mm, 1)
            vector.tensor_copy(
                wsq_row4[:, :, :],
                bass.AP(tensor=ps_w, offset=0, ap=[[NLOC, 2], [0, MT], [1, NLOC]]),
            ).then_inc(s_brd)
            vector.drain()  # DVE RAW: reduces read xsq_scrD
            for m in range(2):
                inst = vector.tensor_reduce(
                    xsq_colD[:, m : m + 1], xsq_scrD[:, m, :],
                    axis=mybir.AxisListType.X, op=mybir.AluOpType.add,
                )
            inst.then_inc(s_colD)

        @block.tensor
        def _(tensor):
            # -0.25*||w||^2 reduce, two identical rows
            tensor.wait_ge(s_sq, 2)
            tensor.matmul(
                ps_w[:, :], lhsT=neg_q[:, :], rhs=wlsq[0][:, :],
                start=True, stop=False,
            )
            tensor.matmul(
                ps_w[:, :], lhsT=neg_q[:, :], rhs=wlsq[1][:, :],
                start=False, stop=True,
            ).then_inc(s_mm)  # = 1
            # main fp16 matmuls: one PSUM bank, single start on the first
            tensor.wait_ge(s_xt, 16)
            for k in range(KT):
                for m in range(MT):
                    tensor.matmul(
                        ps_all[:, m, :],
                        lhsT=xtf_sb[:, k, ts(m, P)],
                        rhs=wlf_sb[:, k, :],
                        start=(k == 0 and m == 0), stop=False,
                        skip_group_check=True,
                    )
            # fold -0.5*||w||^2 per m-slice (K=2: two -0.25 rows)
            tensor.wait_ge(s_brd, 1)
            for m in range(MT):
                tensor.matmul(
                    ps_all[:, m, :],
                    lhsT=ones_m[:, :],
                    rhs=wsq_row4[:, m, :],
                    start=False, stop=True, skip_group_check=True,
                ).then_inc(s_mm)  # = 2 + m

    nc.compile()
    return nc


def _get_nc():
    global _NC
    if _NC is None:
        _NC = _build()
    return _NC


def _make_in_maps(x: np.ndarray, weight: np.ndarray):
    x = np.ascontiguousarray(x.astype(np.float32, copy=False))
    xtf = np.ascontiguousarray(x.T.astype(np.float16))
    wf = weight.astype(np.float16)
    return [
        {
            "xtf": xtf,
            "xn": x,
            "wlf": np.ascontiguousarray(wf[:, c * NLOC : (c + 1) * NLOC]),
        }
        for c in range(NCORES)
    ]


def run(x: np.ndarray, weight: np.ndarray, trace: bool = False):
    """Returns (full_output, BassKernelResults)."""
    from concourse.bass_utils import run_bass_kernel_spmd

    nc = _get_nc()
    res = run_bass_kernel_spmd(
        nc, _make_in_maps(x, weight), core_ids=list(range(NCORES)), trace=trace
    )
    full = np.concatenate(
        [res.results[c]["out"] for c in range(NCORES)], axis=1
    )
    return full, res


def kernel(x: np.ndarray, weight: np.ndarray) -> np.ndarray:
    return run(x, weight)[0]


# revision 35
# speedup vs baseline: 1.1071x; 1.0532x over previous
"""Euclidean distance layer on 8 Trainium2 NeuronCores.

out[b, o] = || x[b, :] - weight[:, o] ||_2
x: [512, 256] f32, weight: [256, 1024] f32 -> out: [512, 1024] f32

Sharding: tensor-parallel over output features (8 x 128 columns per core).

Per core:  dist^2 = -2 * ( x~@w~_loc - 0.5*||w~_loc||^2 ) + ||x||^2
where x~, w~ are fp16 roundings of x, w (the x.w and ||w||^2 terms tolerate
fp16 easily; ||x||^2 stays fp32 -> ~2e-5 relative error on dist).
  - x~@w~ on the PE into one fp32 PSUM bank [128, 4x128]
  - ||w~||^2: fp16 squares (DVE) -> [-0.25] x2-column reduce matmul ->
    stride-0 broadcast row copy -> K=2 fp16 fold matmuls
  - ||x||^2 per-partition fp32 columns, split: batch tiles 0/1 via ACT
    Square+accum_out, tiles 2/3 via DVE mul+reduce
  - final: out = sqrt(-2 * psum + ||x||^2) on ACT (affine scale + bias)
Raw bacc, manual semaphores. Input DMAs issue in the pre-block preamble.
Host work is layout/dtype prep only.
"""

from contextlib import ExitStack

import numpy as np

B = 512      # batch
K = 256      # inputSize (contraction dim)
NOUT = 1024  # outputSize
NCORES = 8
NLOC = NOUT // NCORES  # 128 output features per core
P = 128                # partitions
KT = K // P            # 2 contraction chunks
MT = B // P            # 4 batch tiles

_NC = None  # cached compiled Bass program (same SPMD program on all cores)


def _build():
    import concourse.bass as bass
    from concourse import bacc, mybir

    f32 = mybir.dt.float32
    f16 = mybir.dt.float16
    Sqrt = mybir.ActivationFunctionType.Sqrt
    Square = mybir.ActivationFunctionType.Square
    ts = bass.ts

    nc = bacc.Bacc(
        "TRN2", target_bir_lowering=False, debug=False, num_devices=NCORES
    )

    xtf = nc.dram_tensor("xtf", [K, B], f16, kind="ExternalInput")
    xn = nc.dram_tensor("xn", [B, K], f32, kind="ExternalInput")
    wlf = nc.dram_tensor("wlf", [K, NLOC], f16, kind="ExternalInput")
    out = nc.dram_tensor("out", [B, NLOC], f32, kind="ExternalOutput")

    with ExitStack() as ctx:
        e = ctx.enter_context
        xtf_sb = e(nc.sbuf_tensor("xtfs", [P, KT, B], f16))
        wlf_sb = e(nc.sbuf_tensor("wlfs", [P, KT, NLOC], f16))
        xn_sb = [e(nc.sbuf_tensor(f"xns{h}", [P, 2, K], f32)) for h in range(2)]
        wlsq = [e(nc.sbuf_tensor(f"wlsq{k}", [P, NLOC], f16)) for k in range(KT)]
        xsq_scrA = e(nc.sbuf_tensor("xsqsA", [P, 2, K], f32))
        xsq_scrD = e(nc.sbuf_tensor("xsqsD", [P, 2, K], f32))
        xsq_colA = e(nc.sbuf_tensor("xsqcA", [P, 2], f32))
        xsq_colD = e(nc.sbuf_tensor("xsqcD", [P, 2], f32))
        neg_q = e(nc.sbuf_tensor("neg_q", [P, 2], f16))
        ones_m = e(nc.sbuf_tensor("ones_m", [2, P], f16))
        wsq_row4 = e(nc.sbuf_tensor("wsq_row4", [2, MT, NLOC], f16))
        out_sb = e(nc.sbuf_tensor("out_sb", [P, MT, NLOC], f32))
        actwarm = e(nc.sbuf_tensor("actwarm", [1, 1], f32))

        ps_w = e(nc.psum_tensor("ps_w", [2, NLOC], f32))   # -0.25*||w||^2 x2
        ps_all = e(nc.psum_tensor("ps_all", [P, MT, NLOC], f32))  # one bank

        s_wl = e(nc.semaphore("s_wl"))
        s_xt = e(nc.semaphore("s_xt"))
        s_xn = [e(nc.semaphore(f"s_xn{h}")) for h in range(2)]
        s_sq = e(nc.semaphore("s_sq"))      # 2 = both wlsq done
        s_mm = e(nc.semaphore("s_mm"))      # 1 = wsq reduce, 2+m = aug m
        s_brd = e(nc.semaphore("s_brd"))    # 1 = wsq_row4 broadcast ready
        s_colD = e(nc.semaphore("s_colD"))  # 1 = xsq cols for m2/m3 ready
        s_sqrt = e(nc.semaphore("s_sqrt"))  # m+1 = sqrt tile m in out_sb
        s_out = e(nc.semaphore("s_out"))    # 16 = sync output DMA landed
        s_out2 = e(nc.semaphore("s_out2"))  # 16 = scalar output DMA landed

        block = e(nc.Block())

        @block.sync
        def _(sync):
            sync.dma_start(
                out=xn_sb[0][:, :, :],
                in_=xn[0 : 2 * P, :].rearrange("(c p) k -> p c k", p=P),
            ).then_inc(s_xn[0], 16)
            sync.dma_start(
                out=wlf_sb[:, :, :],
                in_=wlf[:, :].rearrange("(c p) o -> p c o", p=P),
            ).then_inc(s_wl, 16)
            sync.dma_start(
                out=xtf_sb[:, :, :],
                in_=xtf[:, :].rearrange("(c p) b -> p c b", p=P),
            ).then_inc(s_xt, 16)
            sync.wait_ge(s_sqrt, 2)
            sync.dma_start(
                out=out[0 : 2 * P, :].rearrange("(m p) o -> p m o", p=P),
                in_=out_sb[:, 0:2, :],
            ).then_inc(s_out, 16)
            sync.wait_ge(s_out, 16)

        @block.gpsimd
        def _(gpsimd):
            gpsimd.dma_start(
                out=xn_sb[1][:, :, :],
                in_=xn[2 * P : 4 * P, :].rearrange("(c p) k -> p c k", p=P),
            ).then_inc(s_xn[1], 16)
            gpsimd.wait_ge(s_xn[1], 16)

        @block.scalar
        def _(scalar):
            # ||x||^2 for batch tiles 0/1 (fp32, Square + free-dim accum)
            # NOTE: also gated on s_xn[1] -- ACT accum activations racing
            # in-flight DMA traffic crash the exec unit on this stack
            scalar.wait_ge(s_xn[1], 16)
            scalar.wait_ge(s_xn[0], 16)
            for m in range(2):
                scalar.activation(
                    xsq_scrA[:, m, :], xn_sb[0][:, m, :], Square,
                    accum_out=xsq_colA[:, m : m + 1],
                )
            scalar.drain()  # ACT RAW: sqrts below read xsq_colA
            for m in range(MT):
                scalar.wait_ge(s_mm, 2 + m)
                if m == 2:
                    scalar.wait_ge(s_colD, 1)
                bias = (
                    xsq_colA[:, m : m + 1] if m < 2
                    else xsq_colD[:, m - 2 : m - 1]
                )
                scalar.activation(
                    out_sb[:, m, :], ps_all[:, m, :], Sqrt,
                    bias=bias, scale=-2.0,
                ).then_inc(s_sqrt)
            scalar.wait_ge(s_sqrt, MT)
            scalar.dma_start(
                out=out[2 * P : 4 * P, :].rearrange("(m p) o -> p m o", p=P),
                in_=out_sb[:, 2:4, :],
            ).then_inc(s_out2, 16)
            scalar.wait_ge(s_out2, 16)


        @block.vector
        def _(vector):
            vector.memset(neg_q[:, :], -0.25)
            vector.memset(ones_m[:, :], 1.0)
            vector.wait_ge(s_wl, 16)
            vector.tensor_mul(wlsq[0][:, :], wlf_sb[:, 0, :], wlf_sb[:, 0, :])
            vector.tensor_mul(
                wlsq[1][:, :], wlf_sb[:, 1, :], wlf_sb[:, 1, :]
            ).then_inc(s_sq, 2)
            # ||x||^2 for batch tiles 2/3: fp32 squares
            vector.wait_ge(s_xn[1], 16)
            for m in range(2):
                vector.tensor_mul(
                    xsq_scrD[:, m, :], xn_sb[1][:, m, :], xn_sb[1][:, m, :]
                )
            # broadcast -0.25*||w||^2 rows across the 4 m-slices
            vector.wait_ge(s_mm, 1)
            vector.tensor_copy(
                wsq_row4[:, :, :],
                bass.AP(tensor=ps_w, offset=0, ap=[[NLOC, 2], [0, MT], [1, NLOC]]),
            ).then_inc(s_brd)
            vector.drain()  # DVE RAW: reduces read xsq_scrD
            for m in range(2):
                inst = vector.tensor_reduce(
                    xsq_colD[:, m : m + 1], xsq_scrD[:, m, :],
                    axis=mybir.AxisListType.X, op=mybir.AluOpType.add,
                )
            inst.then_inc(s_colD)

        @block.tensor
        def _(tensor):
            # -0.25*||w||^2 reduce, two identical rows
            tensor.wait_ge(s_sq, 2)
            tensor.matmul(
                ps_w[:, :], lhsT=neg_q[:, :], rhs=wlsq[0][:, :],
                start=True, stop=False,
            )
            tensor.matmul(
                ps_w[:, :], lhsT=neg_q[:, :], rhs=wlsq[1][:, :],
                start=False, stop=True,
            ).then_inc(s_mm)  # = 1
            # main fp16 matmuls: one PSUM bank, single start on the first
            tensor.wait_ge(s_xt, 16)
            for k in range(KT):
                for m in range(MT):
                    tensor.matmul(
                        ps_all[:, m, :],
                        lhsT=xtf_sb[:, k, ts(m, P)],
                        rhs=wlf_sb[:, k, :],
                        start=(k == 0 and m == 0), stop=False,
                        skip_group_check=True,
                    )
            # fold -0.5*||w||^2 per m-slice (K=2: two -0.25 rows)
            tensor.wait_ge(s_brd, 1)
            for m in range(MT):
                tensor.matmul(
                    ps_all[:, m, :],
                    lhsT=ones_m[:, :],
                    rhs=wsq_row4[:, m, :],
                    start=False, stop=True, skip_group_check=True,
                ).then_inc(s_mm)  # = 2 + m

    nc.compile()
    return nc


def _get_nc():
    global _NC
    if _NC is None:
        _NC = _build()
    return _NC


def _make_in_maps(x: np.ndarray, weight: np.ndarray):
    x = np.ascontiguousarray(x.astype(np.float32, copy=False))
    xtf = np.ascontiguousarray(x.T.astype(np.float16))
    wf = weight.astype(np.float16)
    return [
        {
            "xtf": xtf,
            "xn": x,
            "wlf": np.ascontiguousarray(wf[:, c * NLOC : (c + 1) * NLOC]),
        }
        for c in range(NCORES)
    ]


def run(x: np.ndarray, weight: np.ndarray, trace: bool = False):
    """Returns (full_output, BassKernelResults)."""
    from concourse.bass_utils import run_bass_kernel_spmd

    nc = _get_nc()
    res = run_bass_kernel_spmd(
        nc, _make_in_maps(x, weight), core_ids=list(range(NCORES)), trace=trace
    )
    full = np.concatenate(
        [res.results[c]["out"] for c in range(NCORES)], axis=1
    )
    return full, res


def kernel(x: np.ndarray, weight: np.ndarray) -> np.ndarray:
    return run(x, weight)[0]
